# revision 23
# baseline (speedup 1.0000x reference)
"""MoE routed dynamics kernel for Trainium2 (8 NeuronCores, expert-parallel).

Problem: for each row b of a [B, D+A] input, route through one of P=8
two-layer MLPs selected by policy_indices[b]:
    h = relu(x @ W1[p] + b1[p]);  y = h @ W2[p] + b2[p]

Sharding: expert-parallel. Core p owns expert p's weights (resident in
SBUF) and processes exactly the rows routed to expert p. The all-to-all
dispatch keyed on policy_indices happens on the host at shard time
(gather rows by expert, pad to a common capacity C = max expert count),
and the inverse scatter happens at unshard time.

All matmul operands are bf16 (host pre-casts); PSUM accumulation stays
fp32 and biases are applied in fp32 — end-to-end rel err ~4e-3 against
the fp32 reference. bf16 halves HBM traffic and LDWEIGHTS time vs
fp32r (PE stream rate is 1 cycle/row for both at these sizes).

DRAM layouts are k-major packed so every logical transfer is ONE large
DMA (HWDGE rings execute DMAs serially per issuing engine):
    xT   [128, KX=5, C]   xT[r,k,c]   = x_pad[k*128+r, c]
    w1   [128, KX=5, H]   w1[r,k,m]   = W1_pad[k*128+r, m]
    w2   [128, KH=8, D]   w2[r,k,m]   = W2[k*128+r, m]
    outT [128, MD=4, C]   outT[r,d,c] = y[d*128+r, c]     (bf16 store)

The startup is HBM-bandwidth-bound (~5 MB of loads at ~360 GB/s), so
ALL DMAs ride the Sync HWDGE ring in compute-need order (the ring is
FIFO, so priority order means backpressure only delays less-urgent
transfers; the Scalar queue carries only relu so ring backpressure
cannot stall PSUM recycling): the first column chunks are small so the
PE starts after ~0.5 MB lands, and L1 runs two chunks ahead of L2 so
the w2 load is well off the critical path. A short burst of throwaway
matmuls on a memset scratch tile bridges the HAM clock-gate ramp
(~3.4us of sustained PE activity lifts the cold 1.2GHz throttle to
2.4GHz) while the first DMAs land.
"""

import math

import numpy as np

_B = 16384
_P = 8
_D = 512
_A = 64
_H = 1024
_DA = _D + _A    # 576
_KX = 5          # ceil(576/128): K-chunks of layer 1 (zero-padded to 640)
_KH = _H // 128  # 8: K-chunks of layer 2
_MH = _H // 128  # 8: output row-tiles of layer 1
_MD = _D // 128  # 4: output row-tiles of layer 2
_N_CORES = 8
_WARM_MM = 18    # PE warmup matmuls: bridges the HAM clock-gate ramp AND
                 # the ~4.5us from ring start to the first weight/x
                 # completion semaphores, so the PE never idles (an idle
                 # gap restarts the 3.4us clock ramp)

_kernel_cache: dict = {}


def _chunks_of(C: int):
    """Small chunks first (fast PE start while DMAs land), then 512s,
    remainder >=128 last (small kernel tail)."""
    chunks = []
    r = C
    for warm in (128, 256):
        if r >= warm + 128:
            chunks.append(warm)
            r -= warm
    while r > 0:
        n = min(512, r)
        if r - n and r - n < 128:
            n = r - 128
        chunks.append(n)
        r -= n
    return chunks


def _build_bass(C: int):
    import concourse.bacc as bacc
    import concourse.mybir as mybir
    from concourse.tile import TileContext

    fp32 = mybir.dt.float32
    bf16 = mybir.dt.bfloat16
    act = mybir.ActivationFunctionType

    assert C % 8 == 0 and C >= 256, C
    nls = _chunks_of(C)
    n0s = [sum(nls[:i]) for i in range(len(nls))]
    n_chunks = list(zip(n0s, nls))
    NC = len(n_chunks)

    nc = bacc.Bacc()
    xT = nc.declare_dram_parameter("xT", [128, _KX, C], bf16, isOutput=False)
    w1 = nc.declare_dram_parameter("w1", [128, _KX, _H], bf16, isOutput=False)
    b1 = nc.declare_dram_parameter("b1", [128, _MH], fp32, isOutput=False)
    w2 = nc.declare_dram_parameter("w2", [128, _KH, _D], bf16, isOutput=False)
    b2 = nc.declare_dram_parameter("b2", [128, _MD], fp32, isOutput=False)
    outT = nc.declare_dram_parameter("outT", [128, _MD, C], bf16, isOutput=True)

    with TileContext(nc) as tc:
        with (
            tc.tile_pool(name="wpool", bufs=1) as wpool,
            tc.tile_pool(name="xpool", bufs=NC) as xpool,
            tc.tile_pool(name="hpool", bufs=3) as hpool,
            tc.tile_pool(name="ypool", bufs=2) as ypool,
            tc.tile_pool(name="ps1", bufs=4, space="PSUM") as ps1,
            tc.tile_pool(name="ps2", bufs=4, space="PSUM") as ps2,
        ):
            # --- PE warmup: throwaway matmuls on a memset scratch tile ---
            # (the memset is the DVE queue's first instruction, so the
            # warmup starts right at engine release; results land in a psum
            # bank that is never read)
            warm = wpool.tile([128, 256], bf16, tag="warm")
            nc.vector.memset(warm[:, :], 0)
            ps_w = ps1.tile([128, 256], fp32, tag="ps1")
            for _ in range(_WARM_MM):
                nc.tensor.matmul(ps_w[:, :], warm[:, 0:128], warm[:, :],
                                 start=True, stop=True)

            # --- ALL DMAs on the Sync HWDGE ring, in compute-need order ---
            # The ring executes FIFO at ~full HBM rate; priority order means
            # ring backpressure only ever delays less-urgent transfers. The
            # Scalar queue carries ONLY the relu ACTIVATEs: a weight DMA
            # issued from nc.scalar blocks relu behind ring backpressure and
            # stalls PSUM recycling (measured 6us PE stall).
            w1_sb = wpool.tile([128, _KX, _H], bf16, tag="w1")
            w2_sb = wpool.tile([128, _KH, _D], bf16, tag="w2")
            b1_sb = wpool.tile([128, _MH], fp32, tag="b1")
            b2_sb = wpool.tile([128, _MD], fp32, tag="b2")
            x_sb = []
            for ci, (n0, nl) in enumerate(n_chunks):
                x_sb.append(xpool.tile([128, _KX, nl], bf16, tag="x", name=f"x{ci}"))

            def ld(out, in_):
                nc.sync.dma_start(out=out, in_=in_)

            # The first three (small) weight loads go on the otherwise-idle
            # Scalar HWDGE ring so their completion sems don't queue behind
            # the x transfers — they all retire before the first relu is
            # enqueued, so they can't block the ACT queue (three small DMAs
            # stay under the ring's backpressure depth).
            nc.scalar.dma_start(out=w1_sb[:, :, 0:256], in_=w1[:, :, 0:256])
            nc.scalar.dma_start(out=b1_sb[:, :], in_=b1[:, :])
            nc.scalar.dma_start(out=w1_sb[:, :, 256:512], in_=w1[:, :, 256:512])

            def ldx(ci):
                n0, nl = n_chunks[ci]
                ld(x_sb[ci][:, :, :], xT[:, :, n0 : n0 + nl])

            ldx(0)
            if NC > 1:
                ldx(1)
            ld(w1_sb[:, :, 512:768], w1[:, :, 512:768])
            ld(w1_sb[:, :, 768:_H], w1[:, :, 768:_H])
            if NC > 2:
                ldx(2)
            ld(w2_sb[:, :, 0:256], w2[:, :, 0:256])
            ld(w2_sb[:, :, 256:_D], w2[:, :, 256:_D])
            # Late-needed loads go via the idle GpSimd SWDGE queue: its slow
            # serial issue (~2us each) naturally staggers them behind the
            # critical w1/x loads, so the early HBM window — and therefore
            # the first weight completion semaphores — isn't delayed by
            # traffic nobody needs yet.
            nc.gpsimd.dma_start(out=b2_sb[:, :], in_=b2[:, :])
            for ci in range(3, NC):
                n0, nl = n_chunks[ci]
                nc.gpsimd.dma_start(out=x_sb[ci][:, :, :], in_=xT[:, :, n0 : n0 + nl])

            def chain(ci, m, h_sb):
                n0, nl = n_chunks[ci]
                ps = ps1.tile([128, nl], fp32, tag="ps1", name=f"ps1_{ci}_{m}")
                for k in range(_KX):
                    nc.tensor.matmul(
                        ps[:, :],
                        w1_sb[:, k, m * 128 : (m + 1) * 128],
                        x_sb[ci][:, k, :],
                        start=(k == 0),
                        stop=(k == _KX - 1),
                    )
                ht = hpool.tile([128, nl], bf16, tag=f"h_{m}", name=f"h_{ci}_{m}")
                nc.scalar.activation(ht[:], ps[:], act.Relu, bias=b1_sb[:, m : m + 1])
                h_sb.append(ht)

            def l1(ci):
                h_sb = []
                for m in range(_MH):
                    chain(ci, m, h_sb)
                return h_sb

            def l2(ci, h_sb):
                n0, nl = n_chunks[ci]
                last = ci == NC - 1
                yt = ypool.tile([128, _MD, nl], bf16, tag="y")
                for d in range(_MD):
                    ps = ps2.tile([128, nl], fp32, tag="ps2")
                    for m in range(_MH):
                        nc.tensor.matmul(
                            ps[:, :],
                            w2_sb[:, m, d * 128 : (d + 1) * 128],
                            h_sb[m][:, :],
                            start=(m == 0),
                            stop=(m == _MH - 1),
                        )
                    # Bias-add on DVE (idle) instead of ACT (busy with relu).
                    nc.vector.tensor_scalar_add(yt[:, d, :], ps[:, :], b2_sb[:, d : d + 1])
                    if last:
                        # Per-d store on the last chunk shrinks the kernel tail.
                        nc.sync.dma_start(
                            out=outT[:, d, n0 : n0 + nl], in_=yt[:, d, :]
                        )
                if not last:
                    nc.sync.dma_start(out=outT[:, :, n0 : n0 + nl], in_=yt[:, :, :])

            # Software pipeline: L1 runs two chunks ahead of L2 (h is
            # triple-buffered), so the w2 load and each x chunk stay well
            # off the PE's critical path. The first two (small) chunks'
            # L1 chains are interleaved by m so each successive w1 slice's
            # first use trails its DMA by ~2 chain times — DMA completion
            # semaphores fire ~2-4us after the data lands at startup.
            if NC >= 2:
                h0, h1 = [], []
                for m in range(_MH):
                    chain(0, m, h0)
                    chain(1, m, h1)
                hq = [h0, h1]
            else:
                hq = [l1(0)]
            for ci in range(2, NC):
                hq.append(l1(ci))
                l2(ci - 2, hq.pop(0))
            base = NC - len(hq)
            for i, h in enumerate(hq):
                l2(base + i, h)

    nc.compile()
    return nc


def _get_bass(C: int):
    nc = _kernel_cache.get(C)
    if nc is None:
        nc = _build_bass(C)
        _kernel_cache[C] = nc
    return nc


def _bf16(a):
    import ml_dtypes

    return np.ascontiguousarray(a.astype(ml_dtypes.bfloat16))


def _pack_k_major(a, kchunks):
    """[K, M] -> [128, kchunks, M] with K zero-padded to kchunks*128."""
    k, m = a.shape
    out = np.zeros((kchunks * 128, m), dtype=a.dtype)
    out[:k] = a
    return np.ascontiguousarray(out.reshape(kchunks, 128, m).transpose(1, 0, 2))


def _prepare_in_maps(latents, actions, policy_indices, W1, b1, W2, b2):
    """Expert-parallel dispatch: returns (in_maps, C, order, offs, counts)."""
    latents = np.asarray(latents, dtype=np.float32)
    actions = np.asarray(actions, dtype=np.float32)
    pi = np.asarray(policy_indices).astype(np.int64)
    W1 = np.asarray(W1, dtype=np.float32)
    b1 = np.asarray(b1, dtype=np.float32)
    W2 = np.asarray(W2, dtype=np.float32)
    b2 = np.asarray(b2, dtype=np.float32)

    B = latents.shape[0]
    counts = np.bincount(pi, minlength=_P)
    order = np.argsort(pi, kind="stable")
    offs = np.concatenate(([0], np.cumsum(counts)))

    # Per-core capacity: the max expert count, rounded up to 8 columns.
    C = max(256, int(math.ceil(counts.max() / 8)) * 8)

    x = np.empty((B, _DA), dtype=np.float32)
    x[:, :_D] = latents
    x[:, _D:] = actions
    x_sorted = x[order]

    in_maps = []
    for p in range(_P):
        xp = np.zeros((_DA, C), dtype=np.float32)
        xp[:, : counts[p]] = x_sorted[offs[p] : offs[p + 1]].T
        in_maps.append(
            {
                "xT": _bf16(_pack_k_major(xp, _KX)),
                "w1": _bf16(_pack_k_major(W1[p], _KX)),
                "b1": np.ascontiguousarray(b1[p].reshape(_MH, 128).T),
                "w2": _bf16(_pack_k_major(W2[p], _KH)),
                "b2": np.ascontiguousarray(b2[p].reshape(_MD, 128).T),
            }
        )
    return in_maps, C, order, offs, counts


def _spot_check(out, order, offs, latents, actions, W1, b1, W2, b2):
    """Recompute a few rows per expert on the host (bf16 model) and flag
    gross corruption (stale tiles / flaky device), not rounding noise."""
    import ml_dtypes

    bf = lambda a: a.astype(ml_dtypes.bfloat16).astype(np.float32)
    for p in range(_P):
        span = order[offs[p] : offs[p + 1]]
        if len(span) == 0:
            continue
        # Sample the start, middle, and end of the expert's column range so
        # every device-side chunk position is represented.
        rows = span[sorted({0, len(span) // 2, len(span) - 1})]
        x = np.concatenate([latents[rows], actions[rows]], axis=1)
        h = bf(np.maximum(bf(x) @ bf(W1[p]) + b1[p], 0.0))
        y = h @ bf(W2[p]) + b2[p]
        got = out[rows]
        if np.linalg.norm(got - y) > 0.1 * (np.linalg.norm(y) + 1e-6):
            return False
    return True


def kernel(latents, actions, policy_indices, W1, b1, W2, b2):
    from concourse.bass_utils import run_bass_kernel_spmd

    in_maps, C, order, offs, counts = _prepare_in_maps(
        latents, actions, policy_indices, W1, b1, W2, b2
    )
    nc = _get_bass(C)

    latents = np.asarray(latents, dtype=np.float32)
    actions = np.asarray(actions, dtype=np.float32)
    B = latents.shape[0]
    out = np.empty((B, _D), dtype=np.float32)
    for _attempt in range(2):
        results = run_bass_kernel_spmd(nc, in_maps, list(range(_N_CORES))).results
        for p in range(_P):
            yT = np.asarray(results[p]["outT"], dtype=np.float32)  # [128, MD, C]
            y = yT.transpose(1, 0, 2).reshape(_D, C)
            out[order[offs[p] : offs[p + 1]]] = y[:, : counts[p]].T
        if _spot_check(out, order, offs, latents, actions,
                       np.asarray(W1, dtype=np.float32), np.asarray(b1, dtype=np.float32),
                       np.asarray(W2, dtype=np.float32), np.asarray(b2, dtype=np.float32)):
            break
    return out


# revision 26
# speedup vs baseline: 1.0083x; 1.0083x over previous
"""MoE routed dynamics kernel for Trainium2 (8 NeuronCores, expert-parallel).

Problem: for each row b of a [B, D+A] input, route through one of P=8
two-layer MLPs selected by policy_indices[b]:
    h = relu(x @ W1[p] + b1[p]);  y = h @ W2[p] + b2[p]

Sharding: expert-parallel. Core p owns expert p's weights (resident in
SBUF) and processes exactly the rows routed to expert p. The all-to-all
dispatch keyed on policy_indices happens on the host at shard time
(gather rows by expert, pad to a common capacity C = max expert count),
and the inverse scatter happens at unshard time.

All matmul operands are bf16 (host pre-casts); PSUM accumulation stays
fp32 and biases are applied in fp32 — end-to-end rel err ~4e-3 against
the fp32 reference. bf16 halves HBM traffic and LDWEIGHTS time vs
fp32r (PE stream rate is 1 cycle/row for both at these sizes).

DRAM layouts are k-major packed so every logical transfer is ONE large
DMA (HWDGE rings execute DMAs serially per issuing engine):
    xT   [128, KX=5, C]   xT[r,k,c]   = x_pad[k*128+r, c]
    w1   [128, KX=5, H]   w1[r,k,m]   = W1_pad[k*128+r, m]
    w2   [128, KH=8, D]   w2[r,k,m]   = W2[k*128+r, m]
    outT [128, MD=4, C]   outT[r,d,c] = y[d*128+r, c]     (bf16 store)

The startup is HBM-bandwidth-bound (~5 MB of loads at ~360 GB/s), so
ALL DMAs ride the Sync HWDGE ring in compute-need order (the ring is
FIFO, so priority order means backpressure only delays less-urgent
transfers; the Scalar queue carries only relu so ring backpressure
cannot stall PSUM recycling): the first column chunks are small so the
PE starts after ~0.5 MB lands, and L1 runs two chunks ahead of L2 so
the w2 load is well off the critical path. A short burst of throwaway
matmuls on a memset scratch tile bridges the HAM clock-gate ramp
(~3.4us of sustained PE activity lifts the cold 1.2GHz throttle to
2.4GHz) while the first DMAs land.
"""

import math

import numpy as np

_B = 16384
_P = 8
_D = 512
_A = 64
_H = 1024
_DA = _D + _A    # 576
_KX = 5          # ceil(576/128): K-chunks of layer 1 (zero-padded to 640)
_KH = _H // 128  # 8: K-chunks of layer 2
_MH = _H // 128  # 8: output row-tiles of layer 1
_MD = _D // 128  # 4: output row-tiles of layer 2
_N_CORES = 8
_WARM_MM = 13    # PE warmup matmuls: bridges the HAM clock-gate ramp AND
                 # the ~3us from ring start to the first weight/x
                 # completion semaphores, so the PE never idles (an idle
                 # gap restarts the 3.4us clock ramp)

_kernel_cache: dict = {}


def _chunks_of(C: int):
    """Small chunks first (fast PE start while DMAs land), then 512s,
    remainder >=128 last (small kernel tail)."""
    chunks = []
    r = C
    for warm in (128, 256):
        if r >= warm + 128:
            chunks.append(warm)
            r -= warm
    while r > 0:
        n = min(512, r)
        if r - n and r - n < 128:
            n = r - 128
        chunks.append(n)
        r -= n
    return chunks


def _build_bass(C: int):
    import concourse.bacc as bacc
    import concourse.mybir as mybir
    from concourse.tile import TileContext

    fp32 = mybir.dt.float32
    bf16 = mybir.dt.bfloat16
    act = mybir.ActivationFunctionType

    assert C % 8 == 0 and C >= 256, C
    nls = _chunks_of(C)
    n0s = [sum(nls[:i]) for i in range(len(nls))]
    n_chunks = list(zip(n0s, nls))
    NC = len(n_chunks)

    nc = bacc.Bacc()
    xT = nc.declare_dram_parameter("xT", [128, _KX, C], bf16, isOutput=False)
    w1 = nc.declare_dram_parameter("w1", [128, _KX, _H], bf16, isOutput=False)
    b1 = nc.declare_dram_parameter("b1", [128, _MH], fp32, isOutput=False)
    w2 = nc.declare_dram_parameter("w2", [128, _KH, _D], bf16, isOutput=False)
    b2 = nc.declare_dram_parameter("b2", [128, _MD], fp32, isOutput=False)
    outT = nc.declare_dram_parameter("outT", [128, _MD, C], bf16, isOutput=True)

    with TileContext(nc) as tc:
        with (
            tc.tile_pool(name="wpool", bufs=1) as wpool,
            tc.tile_pool(name="xpool", bufs=NC) as xpool,
            tc.tile_pool(name="hpool", bufs=3) as hpool,
            tc.tile_pool(name="ypool", bufs=2) as ypool,
            tc.tile_pool(name="ps1", bufs=4, space="PSUM") as ps1,
            tc.tile_pool(name="ps2", bufs=4, space="PSUM") as ps2,
        ):
            # --- PE warmup: throwaway matmuls on a memset scratch tile ---
            # (the memset is the DVE queue's first instruction, so the
            # warmup starts right at engine release; results land in a psum
            # bank that is never read)
            warm = wpool.tile([128, 256], bf16, tag="warm")
            nc.vector.memset(warm[:, :], 0)
            ps_w = ps1.tile([128, 256], fp32, tag="ps1")
            for _ in range(_WARM_MM):
                nc.tensor.matmul(ps_w[:, :], warm[:, 0:128], warm[:, :],
                                 start=True, stop=True)

            # --- ALL DMAs on the Sync HWDGE ring, in compute-need order ---
            # The ring executes FIFO at ~full HBM rate; priority order means
            # ring backpressure only ever delays less-urgent transfers. The
            # Scalar queue carries ONLY the relu ACTIVATEs: a weight DMA
            # issued from nc.scalar blocks relu behind ring backpressure and
            # stalls PSUM recycling (measured 6us PE stall).
            w1_sb = wpool.tile([128, _KX, _H], bf16, tag="w1")
            w2_sb = wpool.tile([128, _KH, _D], bf16, tag="w2")
            b1_sb = wpool.tile([128, _MH], fp32, tag="b1")
            b2_sb = wpool.tile([128, _MD], fp32, tag="b2")
            x_sb = []
            for ci, (n0, nl) in enumerate(n_chunks):
                x_sb.append(xpool.tile([128, _KX, nl], bf16, tag="x", name=f"x{ci}"))

            def ld(out, in_):
                nc.sync.dma_start(out=out, in_=in_)

            # The first (small) weight loads go on the otherwise-idle Scalar
            # HWDGE ring so their completion sems don't queue behind the x
            # transfers — they all retire before the first relu is enqueued,
            # so they can't block the ACT queue (four small DMAs stay under
            # the ring's backpressure depth). w1 m0 ships alone first so the
            # very first L1 chain is gated only by two tiny head-of-queue
            # transfers (x0 on sync, w1m0 here).
            nc.scalar.dma_start(out=w1_sb[:, :, 0:128], in_=w1[:, :, 0:128])
            nc.scalar.dma_start(out=b1_sb[:, :], in_=b1[:, :])
            nc.scalar.dma_start(out=w1_sb[:, :, 128:256], in_=w1[:, :, 128:256])
            nc.scalar.dma_start(out=w1_sb[:, :, 256:512], in_=w1[:, :, 256:512])

            def ldx(ci):
                n0, nl = n_chunks[ci]
                ld(x_sb[ci][:, :, :], xT[:, :, n0 : n0 + nl])

            ldx(0)
            if NC > 1:
                ldx(1)
            ld(w1_sb[:, :, 512:768], w1[:, :, 512:768])
            ld(w1_sb[:, :, 768:_H], w1[:, :, 768:_H])
            if NC > 2:
                ldx(2)
            ld(w2_sb[:, :, 0:256], w2[:, :, 0:256])
            ld(w2_sb[:, :, 256:_D], w2[:, :, 256:_D])
            ld(b2_sb[:, :], b2[:, :])
            for ci in range(3, NC):
                ldx(ci)

            def chain(ci, m, h_sb):
                n0, nl = n_chunks[ci]
                ps = ps1.tile([128, nl], fp32, tag="ps1", name=f"ps1_{ci}_{m}")
                for k in range(_KX):
                    nc.tensor.matmul(
                        ps[:, :],
                        w1_sb[:, k, m * 128 : (m + 1) * 128],
                        x_sb[ci][:, k, :],
                        start=(k == 0),
                        stop=(k == _KX - 1),
                    )
                ht = hpool.tile([128, nl], bf16, tag=f"h_{m}", name=f"h_{ci}_{m}")
                nc.scalar.activation(ht[:], ps[:], act.Relu, bias=b1_sb[:, m : m + 1])
                h_sb.append(ht)

            def l1(ci):
                h_sb = []
                for m in range(_MH):
                    chain(ci, m, h_sb)
                return h_sb

            def l2(ci, h_sb):
                n0, nl = n_chunks[ci]
                last = ci == NC - 1
                yt = ypool.tile([128, _MD, nl], bf16, tag="y")
                for d in range(_MD):
                    ps = ps2.tile([128, nl], fp32, tag="ps2")
                    for m in range(_MH):
                        nc.tensor.matmul(
                            ps[:, :],
                            w2_sb[:, m, d * 128 : (d + 1) * 128],
                            h_sb[m][:, :],
                            start=(m == 0),
                            stop=(m == _MH - 1),
                        )
                    # Bias-add on DVE (idle) instead of ACT (busy with relu).
                    nc.vector.tensor_scalar_add(yt[:, d, :], ps[:, :], b2_sb[:, d : d + 1])
                    if last:
                        # Per-d store on the last chunk shrinks the kernel tail.
                        nc.sync.dma_start(
                            out=outT[:, d, n0 : n0 + nl], in_=yt[:, d, :]
                        )
                if not last:
                    nc.sync.dma_start(out=outT[:, :, n0 : n0 + nl], in_=yt[:, :, :])

            # Software pipeline: L1 runs two chunks ahead of L2 (h is
            # triple-buffered), so the w2 load and each x chunk stay well
            # off the PE's critical path. The first two (small) chunks'
            # L1 chains are interleaved by m so each successive w1 slice's
            # first use trails its DMA by ~2 chain times — DMA completion
            # semaphores fire ~2-4us after the data lands at startup.
            if NC >= 2:
                h0, h1 = [], []
                for m in range(_MH):
                    chain(0, m, h0)
                    chain(1, m, h1)
                hq = [h0, h1]
            else:
                hq = [l1(0)]
            for ci in range(2, NC):
                hq.append(l1(ci))
                l2(ci - 2, hq.pop(0))
            base = NC - len(hq)
            for i, h in enumerate(hq):
                l2(base + i, h)

    nc.compile()
    return nc


def _get_bass(C: int):
    nc = _kernel_cache.get(C)
    if nc is None:
        nc = _build_bass(C)
        _kernel_cache[C] = nc
    return nc


def _bf16(a):
    import ml_dtypes

    return np.ascontiguousarray(a.astype(ml_dtypes.bfloat16))


def _pack_k_major(a, kchunks):
    """[K, M] -> [128, kchunks, M] with K zero-padded to kchunks*128."""
    k, m = a.shape
    out = np.zeros((kchunks * 128, m), dtype=a.dtype)
    out[:k] = a
    return np.ascontiguousarray(out.reshape(kchunks, 128, m).transpose(1, 0, 2))


def _prepare_in_maps(latents, actions, policy_indices, W1, b1, W2, b2):
    """Expert-parallel dispatch: returns (in_maps, C, order, offs, counts)."""
    latents = np.asarray(latents, dtype=np.float32)
    actions = np.asarray(actions, dtype=np.float32)
    pi = np.asarray(policy_indices).astype(np.int64)
    W1 = np.asarray(W1, dtype=np.float32)
    b1 = np.asarray(b1, dtype=np.float32)
    W2 = np.asarray(W2, dtype=np.float32)
    b2 = np.asarray(b2, dtype=np.float32)

    B = latents.shape[0]
    counts = np.bincount(pi, minlength=_P)
    order = np.argsort(pi, kind="stable")
    offs = np.concatenate(([0], np.cumsum(counts)))

    # Per-core capacity: the max expert count, rounded up to 8 columns.
    C = max(256, int(math.ceil(counts.max() / 8)) * 8)

    x = np.empty((B, _DA), dtype=np.float32)
    x[:, :_D] = latents
    x[:, _D:] = actions
    x_sorted = x[order]

    in_maps = []
    for p in range(_P):
        xp = np.zeros((_DA, C), dtype=np.float32)
        xp[:, : counts[p]] = x_sorted[offs[p] : offs[p + 1]].T
        in_maps.append(
            {
                "xT": _bf16(_pack_k_major(xp, _KX)),
                "w1": _bf16(_pack_k_major(W1[p], _KX)),
                "b1": np.ascontiguousarray(b1[p].reshape(_MH, 128).T),
                "w2": _bf16(_pack_k_major(W2[p], _KH)),
                "b2": np.ascontiguousarray(b2[p].reshape(_MD, 128).T),
            }
        )
    return in_maps, C, order, offs, counts


def _spot_check(out, order, offs, latents, actions, W1, b1, W2, b2):
    """Recompute a few rows per expert on the host (bf16 model) and flag
    gross corruption (stale tiles / flaky device), not rounding noise."""
    import ml_dtypes

    bf = lambda a: a.astype(ml_dtypes.bfloat16).astype(np.float32)
    for p in range(_P):
        span = order[offs[p] : offs[p + 1]]
        if len(span) == 0:
            continue
        # Sample the start, middle, and end of the expert's column range so
        # every device-side chunk position is represented.
        rows = span[sorted({0, len(span) // 2, len(span) - 1})]
        x = np.concatenate([latents[rows], actions[rows]], axis=1)
        h = bf(np.maximum(bf(x) @ bf(W1[p]) + b1[p], 0.0))
        y = h @ bf(W2[p]) + b2[p]
        got = out[rows]
        if np.linalg.norm(got - y) > 0.1 * (np.linalg.norm(y) + 1e-6):
            return False
    return True


def kernel(latents, actions, policy_indices, W1, b1, W2, b2):
    from concourse.bass_utils import run_bass_kernel_spmd

    in_maps, C, order, offs, counts = _prepare_in_maps(
        latents, actions, policy_indices, W1, b1, W2, b2
    )
    nc = _get_bass(C)

    latents = np.asarray(latents, dtype=np.float32)
    actions = np.asarray(actions, dtype=np.float32)
    B = latents.shape[0]
    out = np.empty((B, _D), dtype=np.float32)
    for _attempt in range(2):
        results = run_bass_kernel_spmd(nc, in_maps, list(range(_N_CORES))).results
        for p in range(_P):
            yT = np.asarray(results[p]["outT"], dtype=np.float32)  # [128, MD, C]
            y = yT.transpose(1, 0, 2).reshape(_D, C)
            out[order[offs[p] : offs[p + 1]]] = y[:, : counts[p]].T
        if _spot_check(out, order, offs, latents, actions,
                       np.asarray(W1, dtype=np.float32), np.asarray(b1, dtype=np.float32),
                       np.asarray(W2, dtype=np.float32), np.asarray(b2, dtype=np.float32)):
            break
    return out


# revision 28
# speedup vs baseline: 1.0759x; 1.0670x over previous
"""MoE routed dynamics kernel for Trainium2 (8 NeuronCores, expert-parallel).

Problem: for each row b of a [B, D+A] input, route through one of P=8
two-layer MLPs selected by policy_indices[b]:
    h = relu(x @ W1[p] + b1[p]);  y = h @ W2[p] + b2[p]

Sharding: expert-parallel. Core p owns expert p's weights (resident in
SBUF) and processes exactly the rows routed to expert p. The all-to-all
dispatch keyed on policy_indices happens on the host at shard time
(gather rows by expert, pad to a common capacity C = max expert count),
and the inverse scatter happens at unshard time.

All matmul operands are bf16 (host pre-casts); PSUM accumulation stays
fp32 and biases are applied in fp32 — end-to-end rel err ~4e-3 against
the fp32 reference. bf16 halves HBM traffic and LDWEIGHTS time vs
fp32r (PE stream rate is 1 cycle/row for both at these sizes).

DRAM layouts are k-major packed so every logical transfer is ONE large
DMA (HWDGE rings execute DMAs serially per issuing engine):
    xT   [128, KX=5, C]   xT[r,k,c]   = x_pad[k*128+r, c]
    w1   [128, KX=5, H]   w1[r,k,m]   = W1_pad[k*128+r, m]
    w2   [128, KH=8, D]   w2[r,k,m]   = W2[k*128+r, m]
    outT [128, MD=4, C]   outT[r,d,c] = y[d*128+r, c]     (bf16 store)

The startup is HBM-bandwidth-bound (~5 MB of loads at ~360 GB/s), so
ALL DMAs ride the Sync HWDGE ring in compute-need order (the ring is
FIFO, so priority order means backpressure only delays less-urgent
transfers; the Scalar queue carries only relu so ring backpressure
cannot stall PSUM recycling): the first column chunks are small so the
PE starts after ~0.5 MB lands, and L1 runs two chunks ahead of L2 so
the w2 load is well off the critical path. A short burst of throwaway
matmuls on a memset scratch tile bridges the HAM clock-gate ramp
(~3.4us of sustained PE activity lifts the cold 1.2GHz throttle to
2.4GHz) while the first DMAs land.
"""

import math

import numpy as np

_B = 16384
_P = 8
_D = 512
_A = 64
_H = 1024
_DA = _D + _A    # 576
_KX = 5          # ceil(576/128): K-chunks of layer 1 (zero-padded to 640)
_KH = _H // 128  # 8: K-chunks of layer 2
_MH = _H // 128  # 8: output row-tiles of layer 1
_MD = _D // 128  # 4: output row-tiles of layer 2
_N_CORES = 8
_WARM_MM = 18    # PE warmup matmuls: bridges the HAM clock-gate ramp AND
                 # the ~4.5us from ring start to the first weight/x
                 # completion semaphores, so the PE never idles (an idle
                 # gap restarts the 3.4us clock ramp)

_kernel_cache: dict = {}


def _chunks_of(C: int):
    """Small chunks first (fast PE start while DMAs land), then 512s,
    remainder >=128 last (small kernel tail)."""
    chunks = []
    r = C
    for warm in (128, 256):
        if r >= warm + 128:
            chunks.append(warm)
            r -= warm
    while r > 0:
        n = min(512, r)
        if r - n and r - n < 128:
            n = r - 128
        chunks.append(n)
        r -= n
    return chunks


def _build_bass(C: int):
    import concourse.bacc as bacc
    import concourse.mybir as mybir
    from concourse.tile import TileContext

    fp32 = mybir.dt.float32
    bf16 = mybir.dt.bfloat16
    act = mybir.ActivationFunctionType

    assert C % 8 == 0 and C >= 256, C
    nls = _chunks_of(C)
    n0s = [sum(nls[:i]) for i in range(len(nls))]
    n_chunks = list(zip(n0s, nls))
    NC = len(n_chunks)

    nc = bacc.Bacc()
    xT = nc.declare_dram_parameter("xT", [128, _KX, C], bf16, isOutput=False)
    w1 = nc.declare_dram_parameter("w1", [128, _KX, _H], bf16, isOutput=False)
    b1 = nc.declare_dram_parameter("b1", [128, _MH], fp32, isOutput=False)
    w2 = nc.declare_dram_parameter("w2", [128, _KH, _D], bf16, isOutput=False)
    b2 = nc.declare_dram_parameter("b2", [128, _MD], fp32, isOutput=False)
    outT = nc.declare_dram_parameter("outT", [128, _MD, C], bf16, isOutput=True)

    with TileContext(nc) as tc:
        with (
            tc.tile_pool(name="wpool", bufs=1) as wpool,
            tc.tile_pool(name="xpool", bufs=NC) as xpool,
            tc.tile_pool(name="hpool", bufs=3) as hpool,
            tc.tile_pool(name="ypool", bufs=2) as ypool,
            tc.tile_pool(name="ps1", bufs=4, space="PSUM") as ps1,
            tc.tile_pool(name="ps2", bufs=4, space="PSUM") as ps2,
        ):
            # --- PE warmup: throwaway matmuls on a memset scratch tile ---
            # (the memset is the DVE queue's first instruction, so the
            # warmup starts right at engine release; results land in a psum
            # bank that is never read)
            warm = wpool.tile([128, 256], bf16, tag="warm")
            nc.vector.memset(warm[:, :], 0)
            ps_w = ps1.tile([128, 256], fp32, tag="ps1")
            for _ in range(_WARM_MM):
                nc.tensor.matmul(ps_w[:, :], warm[:, 0:128], warm[:, :],
                                 start=True, stop=True)

            # --- ALL DMAs on the Sync HWDGE ring, in compute-need order ---
            # The ring executes FIFO at ~full HBM rate; priority order means
            # ring backpressure only ever delays less-urgent transfers. The
            # Scalar queue carries ONLY the relu ACTIVATEs: a weight DMA
            # issued from nc.scalar blocks relu behind ring backpressure and
            # stalls PSUM recycling (measured 6us PE stall).
            w1_sb = wpool.tile([128, _KX, _H], bf16, tag="w1")
            w2_sb = wpool.tile([128, _KH, _D], bf16, tag="w2")
            b1_sb = wpool.tile([128, _MH], fp32, tag="b1")
            b2_sb = wpool.tile([128, _MD], fp32, tag="b2")
            x_sb = []
            for ci, (n0, nl) in enumerate(n_chunks):
                x_sb.append(xpool.tile([128, _KX, nl], bf16, tag="x", name=f"x{ci}"))

            def ld(out, in_):
                nc.sync.dma_start(out=out, in_=in_)

            # The first (small) weight loads go on the otherwise-idle Scalar
            # HWDGE ring so their completion sems don't queue behind the x
            # transfers — they all retire before the first relu is enqueued,
            # so they can't block the ACT queue (four small DMAs stay under
            # the ring's backpressure depth). w1 m0 ships alone first so the
            # very first L1 chain is gated only by two tiny head-of-queue
            # transfers (x0 on sync, w1m0 here).
            nc.scalar.dma_start(out=w1_sb[:, :, 0:256], in_=w1[:, :, 0:256])
            nc.scalar.dma_start(out=b1_sb[:, :], in_=b1[:, :])
            nc.scalar.dma_start(out=w1_sb[:, :, 256:512], in_=w1[:, :, 256:512])

            def ldx(ci):
                n0, nl = n_chunks[ci]
                ld(x_sb[ci][:, :, :], xT[:, :, n0 : n0 + nl])

            ldx(0)
            if NC > 1:
                ldx(1)
            ld(w1_sb[:, :, 512:768], w1[:, :, 512:768])
            ld(w1_sb[:, :, 768:_H], w1[:, :, 768:_H])
            if NC > 2:
                ldx(2)
            ld(w2_sb[:, :, 0:256], w2[:, :, 0:256])
            ld(w2_sb[:, :, 256:_D], w2[:, :, 256:_D])
            ld(b2_sb[:, :], b2[:, :])
            for ci in range(3, NC):
                ldx(ci)

            def chain(ci, m, h_sb):
                n0, nl = n_chunks[ci]
                ps = ps1.tile([128, nl], fp32, tag="ps1", name=f"ps1_{ci}_{m}")
                for k in range(_KX):
                    nc.tensor.matmul(
                        ps[:, :],
                        w1_sb[:, k, m * 128 : (m + 1) * 128],
                        x_sb[ci][:, k, :],
                        start=(k == 0),
                        stop=(k == _KX - 1),
                    )
                ht = hpool.tile([128, nl], bf16, tag=f"h_{m}", name=f"h_{ci}_{m}")
                nc.scalar.activation(ht[:], ps[:], act.Relu, bias=b1_sb[:, m : m + 1])
                h_sb.append(ht)

            def l1(ci):
                h_sb = []
                for m in range(_MH):
                    chain(ci, m, h_sb)
                return h_sb

            def l2(ci, h_sb):
                n0, nl = n_chunks[ci]
                last = ci == NC - 1
                yt = ypool.tile([128, _MD, nl], bf16, tag="y")
                for d in range(_MD):
                    ps = ps2.tile([128, nl], fp32, tag="ps2")
                    for m in range(_MH):
                        nc.tensor.matmul(
                            ps[:, :],
                            w2_sb[:, m, d * 128 : (d + 1) * 128],
                            h_sb[m][:, :],
                            start=(m == 0),
                            stop=(m == _MH - 1),
                        )
                    # Bias-add on DVE (idle) instead of ACT (busy with relu).
                    nc.vector.tensor_scalar_add(yt[:, d, :], ps[:, :], b2_sb[:, d : d + 1])
                    if last:
                        # Per-d store on the last chunk shrinks the kernel tail.
                        nc.sync.dma_start(
                            out=outT[:, d, n0 : n0 + nl], in_=yt[:, d, :]
                        )
                if not last:
                    nc.sync.dma_start(out=outT[:, :, n0 : n0 + nl], in_=yt[:, :, :])

            # Software pipeline: L1 runs two chunks ahead of L2 (h is
            # triple-buffered), so the w2 load and each x chunk stay well
            # off the PE's critical path. The first two (small) chunks'
            # L1 chains are interleaved by m so each successive w1 slice's
            # first use trails its DMA by ~2 chain times — DMA completion
            # semaphores fire ~2-4us after the data lands at startup.
            if NC >= 2:
                h0, h1 = [], []
                for m in range(_MH):
                    chain(0, m, h0)
                    chain(1, m, h1)
                hq = [h0, h1]
            else:
                hq = [l1(0)]
            for ci in range(2, NC):
                hq.append(l1(ci))
                l2(ci - 2, hq.pop(0))
            base = NC - len(hq)
            for i, h in enumerate(hq):
                l2(base + i, h)

    nc.compile()
    return nc


def _get_bass(C: int):
    nc = _kernel_cache.get(C)
    if nc is None:
        nc = _build_bass(C)
        _kernel_cache[C] = nc
    return nc


def _bf16(a):
    import ml_dtypes

    return np.ascontiguousarray(a.astype(ml_dtypes.bfloat16))


def _pack_k_major(a, kchunks):
    """[K, M] -> [128, kchunks, M] with K zero-padded to kchunks*128."""
    k, m = a.shape
    out = np.zeros((kchunks * 128, m), dtype=a.dtype)
    out[:k] = a
    return np.ascontiguousarray(out.reshape(kchunks, 128, m).transpose(1, 0, 2))


def _prepare_in_maps(latents, actions, policy_indices, W1, b1, W2, b2):
    """Expert-parallel dispatch: returns (in_maps, C, order, offs, counts)."""
    latents = np.asarray(latents, dtype=np.float32)
    actions = np.asarray(actions, dtype=np.float32)
    pi = np.asarray(policy_indices).astype(np.int64)
    W1 = np.asarray(W1, dtype=np.float32)
    b1 = np.asarray(b1, dtype=np.float32)
    W2 = np.asarray(W2, dtype=np.float32)
    b2 = np.asarray(b2, dtype=np.float32)

    B = latents.shape[0]
    counts = np.bincount(pi, minlength=_P)
    order = np.argsort(pi, kind="stable")
    offs = np.concatenate(([0], np.cumsum(counts)))

    # Per-core capacity: the max expert count, rounded up to 8 columns.
    C = max(256, int(math.ceil(counts.max() / 8)) * 8)

    x = np.empty((B, _DA), dtype=np.float32)
    x[:, :_D] = latents
    x[:, _D:] = actions
    x_sorted = x[order]

    in_maps = []
    for p in range(_P):
        xp = np.zeros((_DA, C), dtype=np.float32)
        xp[:, : counts[p]] = x_sorted[offs[p] : offs[p + 1]].T
        in_maps.append(
            {
                "xT": _bf16(_pack_k_major(xp, _KX)),
                "w1": _bf16(_pack_k_major(W1[p], _KX)),
                "b1": np.ascontiguousarray(b1[p].reshape(_MH, 128).T),
                "w2": _bf16(_pack_k_major(W2[p], _KH)),
                "b2": np.ascontiguousarray(b2[p].reshape(_MD, 128).T),
            }
        )
    return in_maps, C, order, offs, counts


def _spot_check(out, order, offs, latents, actions, W1, b1, W2, b2):
    """Recompute a few rows per expert on the host (bf16 model) and flag
    gross corruption (stale tiles / flaky device), not rounding noise."""
    import ml_dtypes

    bf = lambda a: a.astype(ml_dtypes.bfloat16).astype(np.float32)
    for p in range(_P):
        span = order[offs[p] : offs[p + 1]]
        if len(span) == 0:
            continue
        # Sample the start, middle, and end of the expert's column range so
        # every device-side chunk position is represented.
        rows = span[sorted({0, len(span) // 2, len(span) - 1})]
        x = np.concatenate([latents[rows], actions[rows]], axis=1)
        h = bf(np.maximum(bf(x) @ bf(W1[p]) + b1[p], 0.0))
        y = h @ bf(W2[p]) + b2[p]
        got = out[rows]
        if np.linalg.norm(got - y) > 0.1 * (np.linalg.norm(y) + 1e-6):
            return False
    return True


def kernel(latents, actions, policy_indices, W1, b1, W2, b2):
    from concourse.bass_utils import run_bass_kernel_spmd

    in_maps, C, order, offs, counts = _prepare_in_maps(
        latents, actions, policy_indices, W1, b1, W2, b2
    )
    nc = _get_bass(C)

    latents = np.asarray(latents, dtype=np.float32)
    actions = np.asarray(actions, dtype=np.float32)
    B = latents.shape[0]
    out = np.empty((B, _D), dtype=np.float32)
    for _attempt in range(2):
        results = run_bass_kernel_spmd(nc, in_maps, list(range(_N_CORES))).results
        for p in range(_P):
            yT = np.asarray(results[p]["outT"], dtype=np.float32)  # [128, MD, C]
            y = yT.transpose(1, 0, 2).reshape(_D, C)
            out[order[offs[p] : offs[p + 1]]] = y[:, : counts[p]].T
        if _spot_check(out, order, offs, latents, actions,
                       np.asarray(W1, dtype=np.float32), np.asarray(b1, dtype=np.float32),
                       np.asarray(W2, dtype=np.float32), np.asarray(b2, dtype=np.float32)):
            break
    return out
